# revision 28
# baseline (speedup 1.0000x reference)
"""Trainium2 Bass kernel for nn_DiffusionAttentionProcessor.

Joint text+image attention (FLUX-style) with GQA, per-head RMSNorm, RoPE,
joint softmax, and output projections.

Sharding (8 cores): core i owns q-heads 3i..3i+2 and kv-head i (GQA groups
align exactly). Output projections are row-sharded; per-core partial products
are summed on the host (the "all-reduce"), where the biases are also added.

Device-side layout is fully transposed: activations live as [feature, seq]
so every matmul contracts over the partition dim with N=512 moving tiles:
  qT/kT/vT = W_blk^T @ xT           (fp32r, full PE rate)
  RMSNorm over head_dim (= partitions) via ones-vector matmul of squares;
  rsqrt via exp(-0.5*ln(x)) on ACT; per-seq broadcast via rank-1 matmul.
  RoPE pair-swap via a permutation-matrix matmul; cos/sin folded with the
  RMSNorm weights into host-precomputed coefficient arrays.
  scoresT[k,q] = kT_blk^T @ qT      -> exp on ACT (softmax max-term skipped:
                                       scores are bounded ~|10| here)
  outT[d,q]   += V_blk^T @ probsT   (V pre-transposed via PE transpose)
  sums[1,q]   += ones^T @ probsT ; divide outT by sums (broadcast matmul)
  partial[s,n] = outT_blk^T @ Wout  (row-shard), DMA'd out per core.
"""

import numpy as np

import concourse.bass as bass
from concourse import bacc
import concourse.mybir as mybir
import concourse.tile as tile


def _reorder_act_tables():
    """Prefer natural_log_exp_and_others so Ln and Exp share one ACT table set
    (avoids per-call ~1.3us table swaps). Idempotent."""
    if getattr(bacc, "_act_tables_reordered", False):
        return
    orig = bacc.get_activation_tables

    def patched(arch):
        tabs = dict(orig(arch))
        pref = "natural_log_exp_and_others"
        if pref in tabs:
            # Hide Exp from earlier sets so the first covering set for both
            # Ln and Exp is `pref`. Order (= set ids) must not change.
            exp = mybir.ActivationFunctionType.Exp
            for k in tabs:
                if k == pref:
                    break
                if exp in tabs[k]:
                    tabs[k] = tabs[k] - {exp}
        return tabs

    bacc.get_activation_tables = patched
    bacc._act_tables_reordered = True

F32 = mybir.dt.float32
F32R = mybir.dt.float32r
AF = mybir.ActivationFunctionType

S_IMG, S_TXT, D, H, KVH, HD = 4096, 512, 3072, 24, 8, 128
S = S_IMG + S_TXT          # 4608
NCORES = 8
HPC = H // NCORES          # 3 q heads per core
KT = D // 128              # 24 contraction tiles for projections
ST = S // 512              # 9 strips of 512
NB = S // 128              # 36 blocks of 128
SCALE = float(HD) ** -0.5
EPS = 1e-6

_CACHE = {}


def _build():
    _reorder_act_tables()
    nc = bacc.Bacc("TRN2", target_bir_lowering=False)

    xT = nc.dram_tensor("xT", [D, S], F32R, kind="ExternalInput")
    wtxt = nc.dram_tensor("wtxt", [D, 640], F32R, kind="ExternalInput")
    wimg = nc.dram_tensor("wimg", [D, 640], F32R, kind="ExternalInput")
    biasd = nc.dram_tensor("biasd", [2, 5, 128], F32, kind="ExternalInput")
    ropeCq = nc.dram_tensor("ropeCq", [128, S], F32, kind="ExternalInput")
    ropeBq = nc.dram_tensor("ropeBq", [128, S], F32, kind="ExternalInput")
    ropeCk = nc.dram_tensor("ropeCk", [128, S], F32, kind="ExternalInput")
    ropeBk = nc.dram_tensor("ropeBk", [128, S], F32, kind="ExternalInput")
    permd = nc.dram_tensor("permd", [128, 128], F32R, kind="ExternalInput")
    identd = nc.dram_tensor("identd", [128, 128], F32, kind="ExternalInput")
    wout = nc.dram_tensor("wout", [HPC * 128, D], F32R, kind="ExternalInput")
    waout = nc.dram_tensor("waout", [HPC * 128, D], F32R, kind="ExternalInput")
    part = nc.dram_tensor("part", [S, D], F32, kind="ExternalOutput")

    with tile.TileContext(nc) as tc:
        with tc.tile_pool(name="dram", bufs=1, space="DRAM") as dpool, \
             tc.tile_pool(name="const", bufs=1) as cpool:
            # DRAM intermediates (tracked by Tile for RAW deps)
            qk_d = dpool.tile([4 * 128, S], F32R)     # q1,q2,q3,k roped+normed
            v_d = dpool.tile([S, 128], F32R)          # V in natural layout
            oT_d = dpool.tile([HPC * 128, S], F32R)   # attention outT per head

            bias_sb = cpool.tile([128, 2, 5], F32)
            nc.sync.dma_start(bias_sb, biasd.rearrange("t o p -> p t o"))
            perm_sb = cpool.tile([128, 128], F32R)
            nc.sync.dma_start(perm_sb, permd[:, :])
            ident_sb = cpool.tile([128, 128], F32)
            nc.sync.dma_start(ident_sb, identd[:, :])
            ones_col_f = cpool.tile([128, 1], F32)
            nc.any.memset(ones_col_f, 1.0)
            ones_col = cpool.tile([128, 1], F32R)
            nc.vector.tensor_copy(ones_col, ones_col_f)
            ones_row_f = cpool.tile([1, 128], F32)
            nc.any.memset(ones_row_f, 1.0)
            ones_row = cpool.tile([1, 128], F32R)
            nc.vector.tensor_copy(ones_row, ones_row_f)
            eps_sb = cpool.tile([1, 1], F32)
            nc.any.memset(eps_sb, EPS)

            # ---------------- Phase 1: QKV projections + norm + rope -------
            with nc.named_scope("p1"), \
                 tc.tile_pool(name="wq", bufs=1) as wpool, \
                 tc.tile_pool(name="xs", bufs=8) as xpool, \
                 tc.tile_pool(name="rp", bufs=3) as rpool, \
                 tc.tile_pool(name="ev", bufs=2) as epool, \
                 tc.tile_pool(name="acc", bufs=1, space="PSUM") as apool, \
                 tc.tile_pool(name="aux", bufs=1, space="PSUM") as xps:
                wimg_sb = wpool.tile([128, KT, 640], F32R)
                nc.sync.dma_start(wimg_sb, wimg.rearrange("(k p) c -> p k c", p=128))
                wtxt_r = wtxt.rearrange("(k p) c -> p k c", p=128)

                for s in range(ST):
                    tsel = 0 if s == 0 else 1
                    sl = slice(s * 512, (s + 1) * 512)
                    ropes = {}
                    for nm, dr in (("cq", ropeCq), ("bq", ropeBq),
                                   ("ck", ropeCk), ("bk", ropeBk)):
                        t = rpool.tile([128, 512], F32, name=f"rope_{nm}_{s}", tag=nm)
                        nc.sync.dma_start(t, dr[:, sl])
                        ropes[nm] = t

                    accs = [apool.tile([128, 512], F32, name=f"acc{o}_{s}", tag=f"acc{o}")
                            for o in range(5)]
                    for k in range(KT):
                        xt = xpool.tile([128, 512], F32R, name=f"x_{s}_{k}", tag="x")
                        nc.sync.dma_start(xt, xT[k * 128:(k + 1) * 128, sl])
                        if s == 0:
                            wk = xpool.tile([128, 640], F32R, name=f"wt_{k}", tag="wt")
                            nc.sync.dma_start(wk, wtxt_r[:, k, :])
                        else:
                            wk = wimg_sb[:, k, :]
                        for o in range(5):
                            nc.tensor.matmul(
                                accs[o],
                                wk[:, o * 128:(o + 1) * 128],
                                xt,
                                start=(k == 0), stop=(k == KT - 1))

                    # bias-evacuate all five outputs (frees the accumulators)
                    raws = []
                    for o in range(4):
                        raw = epool.tile([128, 512], F32R, name=f"raw{o}_{s}", tag=f"raw{o}")
                        nc.vector.tensor_scalar_add(raw, accs[o], bias_sb[:, tsel, o:o + 1])
                        raws.append(raw)
                    vraw = epool.tile([128, 512], F32, name=f"vraw_{s}", tag="vraw")
                    nc.vector.tensor_scalar_add(vraw, accs[4], bias_sb[:, tsel, 4:5])

                    # RMSNorm stats for q1,q2,q3,k batched: one Ln + one Exp
                    ssqs = epool.tile([1, 2048], F32, name=f"ssqs_{s}", tag="ssqs", bufs=1)
                    for o in range(4):
                        sq = epool.tile([128, 512], F32R, name=f"sq{o}_{s}", tag="sq")
                        nc.vector.tensor_mul(sq, raws[o], raws[o])
                        ssq = xps.tile([1, 512], F32, name=f"ssq{o}_{s}", tag="ssq", bufs=2)
                        nc.tensor.matmul(ssq, ones_col, sq, start=True, stop=True)
                        nc.scalar.copy(ssqs[:, o * 512:(o + 1) * 512], ssq)
                    lnv = epool.tile([1, 2048], F32, name=f"ln_{s}", tag="lnv", bufs=1)
                    nc.scalar.activation(lnv, ssqs, AF.Ln, scale=1.0 / HD, bias=eps_sb)
                    rstd = epool.tile([1, 2048], F32, name=f"rstd_{s}", tag="rstd", bufs=1)
                    nc.scalar.activation(rstd, lnv, AF.Exp, scale=-0.5)

                    for o in range(4):
                        cn, bn = ("cq", "bq") if o < 3 else ("ck", "bk")
                        raw = raws[o]
                        rb = epool.tile([128, 512], F32, name=f"rb{o}_{s}", tag="rb")
                        nc.gpsimd.partition_broadcast(rb, rstd[0:1, o * 512:(o + 1) * 512])
                        qs = xps.tile([128, 512], F32, name=f"qs{o}_{s}", tag="qs")
                        nc.tensor.matmul(qs, perm_sb, raw, start=True, stop=True)
                        t1 = epool.tile([128, 512], F32, name=f"t1{o}_{s}", tag="t1")
                        nc.vector.tensor_mul(t1, raw, ropes[cn])
                        t2 = epool.tile([128, 512], F32, name=f"t2{o}_{s}", tag="t2")
                        nc.vector.tensor_mul(t2, qs, ropes[bn])
                        t3 = epool.tile([128, 512], F32, name=f"t3{o}_{s}", tag="t3")
                        nc.vector.tensor_add(t3, t1, t2)
                        qfin = epool.tile([128, 512], F32R, name=f"qf{o}_{s}", tag="qfin")
                        nc.vector.tensor_mul(qfin, t3, rb)
                        nc.gpsimd.dma_start(qk_d[o * 128:(o + 1) * 128, sl], qfin)

                    # V: transpose to natural [s, d] layout
                    for b in range(4):
                        vt = xps.tile([128, 128], F32, name=f"vt_{s}_{b}", tag="ssq", bufs=2)
                        nc.tensor.transpose(vt, vraw[:, b * 128:(b + 1) * 128], ident_sb)
                        vblk = epool.tile([128, 128], F32R, name=f"vb_{s}_{b}", tag="vblk")
                        nc.vector.tensor_copy(vblk, vt)
                        r0 = (s * 4 + b) * 128
                        nc.gpsimd.dma_start(v_d[r0:r0 + 128, :], vblk)

            # ---------------- Phase 2+3: SDPA + output projections ---------
            with nc.named_scope("p2"), \
                 tc.tile_pool(name="kv", bufs=1) as kvpool, \
                 tc.tile_pool(name="qh", bufs=2) as qpool, \
                 tc.tile_pool(name="pb", bufs=3) as ppool, \
                 tc.tile_pool(name="sm", bufs=2) as smpool, \
                 tc.tile_pool(name="wo", bufs=1) as wopool, \
                 tc.tile_pool(name="ot", bufs=3) as otpool, \
                 tc.tile_pool(name="os", bufs=3) as ospool, \
                 tc.tile_pool(name="sc", bufs=2, space="PSUM") as scps, \
                 tc.tile_pool(name="oa", bufs=1, space="PSUM") as oaps:
                kT_sb = kvpool.tile([128, S], F32R)
                v_sb = kvpool.tile([128, NB, 128], F32R)
                v_r = v_d.rearrange("(t p) d -> p t d", p=128)
                for s in range(ST):
                    ssl = slice(s * 512, (s + 1) * 512)
                    nc.sync.dma_start(kT_sb[:, ssl], qk_d[3 * 128:4 * 128, ssl])
                    nc.sync.dma_start(v_sb[:, 4 * s:4 * s + 4, :], v_r[:, 4 * s:4 * s + 4, :])
                wout_sb = wopool.tile([128, HPC, D], F32R)
                nc.sync.dma_start(wout_sb, wout.rearrange("(hb p) n -> p hb n", p=128))
                waout_sb = wopool.tile([128, HPC, D], F32R)
                nc.sync.dma_start(waout_sb, waout.rearrange("(hb p) n -> p hb n", p=128))

                for h in range(HPC):
                    q_sb = qpool.tile([128, S], F32R, name=f"q_{h}", tag="qh")
                    for s in range(ST):
                        ssl = slice(s * 512, (s + 1) * 512)
                        nc.sync.dma_start(q_sb[:, ssl], qk_d[h * 128:(h + 1) * 128, ssl])
                    for s in range(ST):
                        sl = slice(s * 512, (s + 1) * 512)
                        outacc = oaps.tile([128, 512], F32, name=f"oa_{h}_{s}", tag="oa", bufs=1)
                        sumacc = oaps.tile([1, 512], F32, name=f"sa_{h}_{s}", tag="sa", bufs=1)
                        GK = 2
                        for g in range(NB // GK):
                            sc = scps.tile([128, GK * 512], F32, name=f"sc_{h}_{s}_{g}", tag="sc", bufs=3)
                            for j in range(GK):
                                t = g * GK + j
                                nc.tensor.matmul(
                                    sc[:, j * 512:(j + 1) * 512],
                                    kT_sb[:, t * 128:(t + 1) * 128],
                                    q_sb[:, sl],
                                    start=True, stop=True)
                            pb = ppool.tile([128, GK * 512], F32R, name=f"pb_{h}_{s}_{g}", tag="pb", bufs=4)
                            nc.scalar.activation(pb, sc, AF.Exp, scale=SCALE)
                            for j in range(GK):
                                t = g * GK + j
                                nc.tensor.matmul(
                                    outacc, v_sb[:, t, :],
                                    pb[:, j * 512:(j + 1) * 512],
                                    start=(t == 0), stop=(t == NB - 1))
                            pbf = ppool.tile([128, 512], F32R, name=f"pbf_{h}_{s}_{g}",
                                             tag="pbf", bufs=2)
                            nc.vector.tensor_add(pbf, pb[:, 0:512], pb[:, 512:1024])
                            nc.tensor.matmul(sumacc, ones_col, pbf,
                                             start=(g == 0), stop=(g == NB // GK - 1))
                        oevac = smpool.tile([128, 512], F32, name=f"oe_{h}_{s}", tag="oevac")
                        nc.vector.tensor_copy(oevac, outacc)
                        sum_sb = smpool.tile([1, 512], F32, name=f"su_{h}_{s}", tag="sum_sb")
                        nc.vector.tensor_copy(sum_sb, sumacc)
                        recip = smpool.tile([1, 512], F32, name=f"rc_{h}_{s}", tag="recip")
                        rscr = smpool.tile([1, 512], F32, name=f"rs_{h}_{s}", tag="rscr")
                        nc.vector.reciprocal_approx_accurate(recip, sum_sb, scratch=rscr)
                        rbs = smpool.tile([128, 512], F32, name=f"rbs_{h}_{s}", tag="rbs")
                        nc.gpsimd.partition_broadcast(rbs, recip)
                        ofin = smpool.tile([128, 512], F32R, name=f"of_{h}_{s}", tag="ofin")
                        nc.vector.tensor_mul(ofin, oevac, rbs)
                        nc.gpsimd.dma_start(oT_d[h * 128:(h + 1) * 128, sl], ofin)

                # ---- output projections (interleave with SDPA tail) ----
                oT_r = oT_d.rearrange("(hb p) s -> p hb s", p=128)
                with nc.named_scope("p3"):
                    for sb in range(NB):
                        wsel = waout_sb if sb < (S_TXT // 128) else wout_sb
                        ot = otpool.tile([128, HPC, 128], F32R, name=f"ot_{sb}", tag="ots")
                        nc.sync.dma_start(ot, oT_r[:, :, sb * 128:(sb + 1) * 128])
                        for n in range(D // 512):
                            # late tiles run after SDPA ends: rotate through the
                            # freed oa/sa banks for deeper psum pipelining
                            if sb >= 24:
                                tg = ("sc", "oa", "sa")[n % 3]
                                pool_ = scps if tg == "sc" else oaps
                                ps = pool_.tile([128, 512], F32, name=f"po_{sb}_{n}", tag=tg,
                                                bufs=3 if tg == "sc" else 1)
                            else:
                                ps = scps.tile([128, 512], F32, name=f"po_{sb}_{n}", tag="sc", bufs=3)
                            for hb in range(HPC):
                                nc.tensor.matmul(
                                    ps,
                                    ot[:, hb, :],
                                    wsel[:, hb, n * 512:(n + 1) * 512],
                                    start=(hb == 0), stop=(hb == HPC - 1))
                            osb = ospool.tile([128, 512], F32, name=f"os_{sb}_{n}", tag="os")
                            if n % 2 == 0:
                                nc.vector.tensor_copy(osb, ps)
                            else:
                                nc.scalar.copy(osb, ps)
                            nc.gpsimd.dma_start(
                                part[sb * 128:(sb + 1) * 128, n * 512:(n + 1) * 512], osb)

    nc.compile()
    return nc


def _prep_inputs(inputs):
    """Host-side shard prep: one in_map per core."""
    hs = np.asarray(inputs["hidden_states"], dtype=np.float32)[0]        # [4096, 3072]
    ehs = np.asarray(inputs["encoder_hidden_states"], dtype=np.float32)[0]  # [512, 3072]
    ire = np.asarray(inputs["image_rotary_emb"], dtype=np.float32)       # [2, 4608, 128]

    x = np.concatenate([ehs, hs], axis=0)                 # [S, D], text first
    xT = np.ascontiguousarray(x.T)                        # [D, S]

    cos, sin = ire[0], ire[1]                             # [S, 128]
    nq = np.asarray(inputs["nq"], np.float32)
    nk = np.asarray(inputs["nk"], np.float32)
    anq = np.asarray(inputs["anq"], np.float32)
    ank = np.asarray(inputs["ank"], np.float32)
    wq_rows = np.concatenate([np.broadcast_to(anq, (S_TXT, HD)),
                              np.broadcast_to(nq, (S_IMG, HD))], axis=0)  # [S,128]
    wk_rows = np.concatenate([np.broadcast_to(ank, (S_TXT, HD)),
                              np.broadcast_to(nk, (S_IMG, HD))], axis=0)

    def rope_cb(w_rows):
        C = w_rows * cos
        B = np.empty_like(C)
        B[:, 0::2] = -sin[:, 0::2] * w_rows[:, 1::2]
        B[:, 1::2] = sin[:, 1::2] * w_rows[:, 0::2]
        return (np.ascontiguousarray(C.T), np.ascontiguousarray(B.T))

    Cq, Bq = rope_cb(wq_rows)
    Ck, Bk = rope_cb(wk_rows)

    perm = np.zeros((128, 128), np.float32)
    idx = np.arange(0, 128, 2)
    perm[idx, idx + 1] = 1.0
    perm[idx + 1, idx] = 1.0
    ident = np.eye(128, dtype=np.float32)

    Wq, Wk, Wv = inputs["Wq"], inputs["Wk"], inputs["Wv"]
    aWq, aWk, aWv = inputs["aWq"], inputs["aWk"], inputs["aWv"]
    bq, bk, bv = inputs["bq"], inputs["bk"], inputs["bv"]
    abq, abk, abv = inputs["abq"], inputs["abk"], inputs["abv"]
    Wout, Waout = inputs["Wout"], inputs["Waout"]

    in_maps = []
    for i in range(NCORES):
        qs = slice(i * 384, (i + 1) * 384)
        ks = slice(i * 128, (i + 1) * 128)
        wimg_i = np.ascontiguousarray(np.concatenate(
            [Wq[:, qs], Wk[:, ks], Wv[:, ks]], axis=1), dtype=np.float32)
        wtxt_i = np.ascontiguousarray(np.concatenate(
            [aWq[:, qs], aWk[:, ks], aWv[:, ks]], axis=1), dtype=np.float32)
        bias_i = np.stack([
            np.concatenate([abq[qs].reshape(3, 128), abk[ks][None], abv[ks][None]]),
            np.concatenate([bq[qs].reshape(3, 128), bk[ks][None], bv[ks][None]]),
        ]).astype(np.float32)                              # [2, 5, 128]
        in_maps.append({
            "xT": xT,
            "wtxt": wtxt_i,
            "wimg": wimg_i,
            "biasd": bias_i,
            "ropeCq": Cq, "ropeBq": Bq, "ropeCk": Ck, "ropeBk": Bk,
            "permd": perm, "identd": ident,
            "wout": np.ascontiguousarray(Wout[qs, :], dtype=np.float32),
            "waout": np.ascontiguousarray(Waout[qs, :], dtype=np.float32),
        })
    return in_maps


def kernel(**inputs):
    from concourse.bass_utils import run_bass_kernel_spmd

    if "nc" not in _CACHE:
        _CACHE["nc"] = _build()
    nc = _CACHE["nc"]

    in_maps = _prep_inputs(inputs)
    res = None
    last_err = None
    for _attempt in range(3):
        try:
            res = run_bass_kernel_spmd(nc, in_maps, core_ids=list(range(NCORES)))
            break
        except Exception as e:  # transient device wedges (NRT_EXEC_UNIT_*)
            last_err = e
            try:
                import jax
                jax.clear_backends()
            except Exception:
                pass
    if res is None:
        raise last_err
    total = res.results[0]["part"].astype(np.float64)
    for r in res.results[1:]:
        total += r["part"].astype(np.float64)

    enc = total[:S_TXT] + np.asarray(inputs["baout"], np.float64)
    hid = total[S_TXT:] + np.asarray(inputs["bout"], np.float64)
    return (hid[None].astype(np.float32), enc[None].astype(np.float32))


# revision 29
# speedup vs baseline: 1.0111x; 1.0111x over previous
"""Trainium2 Bass kernel for nn_DiffusionAttentionProcessor.

Joint text+image attention (FLUX-style) with GQA, per-head RMSNorm, RoPE,
joint softmax, and output projections.

Sharding (8 cores): core i owns q-heads 3i..3i+2 and kv-head i (GQA groups
align exactly). Output projections are row-sharded; per-core partial products
are summed on the host (the "all-reduce"), where the biases are also added.

Device-side layout is fully transposed: activations live as [feature, seq]
so every matmul contracts over the partition dim with N=512 moving tiles:
  qT/kT/vT = W_blk^T @ xT           (fp32r, full PE rate)
  RMSNorm over head_dim (= partitions) via ones-vector matmul of squares;
  rsqrt via exp(-0.5*ln(x)) on ACT; per-seq broadcast via rank-1 matmul.
  RoPE pair-swap via a permutation-matrix matmul; cos/sin folded with the
  RMSNorm weights into host-precomputed coefficient arrays.
  scoresT[k,q] = kT_blk^T @ qT      -> exp on ACT (softmax max-term skipped:
                                       scores are bounded ~|10| here)
  outT[d,q]   += V_blk^T @ probsT   (V pre-transposed via PE transpose)
  sums[1,q]   += ones^T @ probsT ; divide outT by sums (broadcast matmul)
  partial[s,n] = outT_blk^T @ Wout  (row-shard), DMA'd out per core.
"""

import numpy as np

import concourse.bass as bass
from concourse import bacc
import concourse.mybir as mybir
import concourse.tile as tile


def _reorder_act_tables():
    """Prefer natural_log_exp_and_others so Ln and Exp share one ACT table set
    (avoids per-call ~1.3us table swaps). Idempotent."""
    if getattr(bacc, "_act_tables_reordered", False):
        return
    orig = bacc.get_activation_tables

    def patched(arch):
        tabs = dict(orig(arch))
        pref = "natural_log_exp_and_others"
        if pref in tabs:
            # Hide Exp from earlier sets so the first covering set for both
            # Ln and Exp is `pref`. Order (= set ids) must not change.
            exp = mybir.ActivationFunctionType.Exp
            for k in tabs:
                if k == pref:
                    break
                if exp in tabs[k]:
                    tabs[k] = tabs[k] - {exp}
        return tabs

    bacc.get_activation_tables = patched
    bacc._act_tables_reordered = True

F32 = mybir.dt.float32
F32R = mybir.dt.float32r
AF = mybir.ActivationFunctionType

S_IMG, S_TXT, D, H, KVH, HD = 4096, 512, 3072, 24, 8, 128
S = S_IMG + S_TXT          # 4608
NCORES = 8
HPC = H // NCORES          # 3 q heads per core
KT = D // 128              # 24 contraction tiles for projections
ST = S // 512              # 9 strips of 512
NB = S // 128              # 36 blocks of 128
SCALE = float(HD) ** -0.5
EPS = 1e-6

_CACHE = {}


def _build():
    _reorder_act_tables()
    nc = bacc.Bacc("TRN2", target_bir_lowering=False)

    xT = nc.dram_tensor("xT", [D, S], F32R, kind="ExternalInput")
    wtxt = nc.dram_tensor("wtxt", [D, 640], F32R, kind="ExternalInput")
    wimg = nc.dram_tensor("wimg", [D, 640], F32R, kind="ExternalInput")
    biasd = nc.dram_tensor("biasd", [2, 5, 128], F32, kind="ExternalInput")
    ropeCq = nc.dram_tensor("ropeCq", [128, S], F32, kind="ExternalInput")
    ropeBq = nc.dram_tensor("ropeBq", [128, S], F32, kind="ExternalInput")
    ropeCk = nc.dram_tensor("ropeCk", [128, S], F32, kind="ExternalInput")
    ropeBk = nc.dram_tensor("ropeBk", [128, S], F32, kind="ExternalInput")
    permd = nc.dram_tensor("permd", [128, 128], F32R, kind="ExternalInput")
    identd = nc.dram_tensor("identd", [128, 128], F32, kind="ExternalInput")
    wout = nc.dram_tensor("wout", [HPC * 128, D], F32R, kind="ExternalInput")
    waout = nc.dram_tensor("waout", [HPC * 128, D], F32R, kind="ExternalInput")
    part = nc.dram_tensor("part", [S, D], F32, kind="ExternalOutput")

    with tile.TileContext(nc) as tc:
        with tc.tile_pool(name="dram", bufs=1, space="DRAM") as dpool, \
             tc.tile_pool(name="const", bufs=1) as cpool:
            # DRAM intermediates (tracked by Tile for RAW deps)
            qk_d = dpool.tile([4 * 128, S], F32R)     # q1,q2,q3,k roped+normed
            v_d = dpool.tile([S, 128], F32R)          # V in natural layout
            oT_d = dpool.tile([HPC * 128, S], F32R)   # attention outT per head

            bias_sb = cpool.tile([128, 2, 5], F32)
            nc.sync.dma_start(bias_sb, biasd.rearrange("t o p -> p t o"))
            perm_sb = cpool.tile([128, 128], F32R)
            nc.sync.dma_start(perm_sb, permd[:, :])
            ident_sb = cpool.tile([128, 128], F32)
            nc.sync.dma_start(ident_sb, identd[:, :])
            ones_col_f = cpool.tile([128, 1], F32)
            nc.any.memset(ones_col_f, 1.0)
            ones_col = cpool.tile([128, 1], F32R)
            nc.vector.tensor_copy(ones_col, ones_col_f)
            ones_row_f = cpool.tile([1, 128], F32)
            nc.any.memset(ones_row_f, 1.0)
            ones_row = cpool.tile([1, 128], F32R)
            nc.vector.tensor_copy(ones_row, ones_row_f)
            eps_sb = cpool.tile([1, 1], F32)
            nc.any.memset(eps_sb, EPS)

            # ---------------- Phase 1: QKV projections + norm + rope -------
            with nc.named_scope("p1"), \
                 tc.tile_pool(name="wq", bufs=1) as wpool, \
                 tc.tile_pool(name="xs", bufs=6) as xpool, \
                 tc.tile_pool(name="rp", bufs=3) as rpool, \
                 tc.tile_pool(name="ev", bufs=2) as epool, \
                 tc.tile_pool(name="acc", bufs=1, space="PSUM") as apool, \
                 tc.tile_pool(name="aux", bufs=1, space="PSUM") as xps:
                wimg_sb = wpool.tile([128, KT, 640], F32R)
                nc.sync.dma_start(wimg_sb, wimg.rearrange("(k p) c -> p k c", p=128))
                wtxt_r = wtxt.rearrange("(k p) c -> p k c", p=128)

                for s in range(ST):
                    tsel = 0 if s == 0 else 1
                    sl = slice(s * 512, (s + 1) * 512)
                    ropes = {}
                    for nm, dr in (("cq", ropeCq), ("bq", ropeBq),
                                   ("ck", ropeCk), ("bk", ropeBk)):
                        t = rpool.tile([128, 512], F32, name=f"rope_{nm}_{s}", tag=nm)
                        nc.sync.dma_start(t, dr[:, sl])
                        ropes[nm] = t

                    accs = [apool.tile([128, 512], F32, name=f"acc{o}_{s}", tag=f"acc{o}")
                            for o in range(5)]
                    for k in range(KT):
                        xt = xpool.tile([128, 512], F32R, name=f"x_{s}_{k}", tag="x")
                        nc.sync.dma_start(xt, xT[k * 128:(k + 1) * 128, sl])
                        if s == 0:
                            wk = xpool.tile([128, 640], F32R, name=f"wt_{k}", tag="wt")
                            nc.sync.dma_start(wk, wtxt_r[:, k, :])
                        else:
                            wk = wimg_sb[:, k, :]
                        for o in range(5):
                            nc.tensor.matmul(
                                accs[o],
                                wk[:, o * 128:(o + 1) * 128],
                                xt,
                                start=(k == 0), stop=(k == KT - 1))

                    # bias-evacuate all five outputs (frees the accumulators)
                    raws = []
                    for o in range(4):
                        raw = epool.tile([128, 512], F32R, name=f"raw{o}_{s}", tag=f"raw{o}")
                        nc.vector.tensor_scalar_add(raw, accs[o], bias_sb[:, tsel, o:o + 1])
                        raws.append(raw)
                    vraw = epool.tile([128, 512], F32, name=f"vraw_{s}", tag="vraw")
                    nc.vector.tensor_scalar_add(vraw, accs[4], bias_sb[:, tsel, 4:5])

                    # RMSNorm stats for q1,q2,q3,k batched: one Ln + one Exp
                    ssqs = epool.tile([1, 2048], F32, name=f"ssqs_{s}", tag="ssqs", bufs=1)
                    for o in range(4):
                        sq = epool.tile([128, 512], F32R, name=f"sq{o}_{s}", tag="sq")
                        nc.vector.tensor_mul(sq, raws[o], raws[o])
                        ssq = xps.tile([1, 512], F32, name=f"ssq{o}_{s}", tag="ssq", bufs=2)
                        nc.tensor.matmul(ssq, ones_col, sq, start=True, stop=True)
                        nc.scalar.copy(ssqs[:, o * 512:(o + 1) * 512], ssq)
                    lnv = epool.tile([1, 2048], F32, name=f"ln_{s}", tag="lnv", bufs=1)
                    nc.scalar.activation(lnv, ssqs, AF.Ln, scale=1.0 / HD, bias=eps_sb)
                    rstd = epool.tile([1, 2048], F32, name=f"rstd_{s}", tag="rstd", bufs=1)
                    nc.scalar.activation(rstd, lnv, AF.Exp, scale=-0.5)

                    for o in range(4):
                        cn, bn = ("cq", "bq") if o < 3 else ("ck", "bk")
                        raw = raws[o]
                        rb = epool.tile([128, 512], F32, name=f"rb{o}_{s}", tag="rb")
                        nc.gpsimd.partition_broadcast(rb, rstd[0:1, o * 512:(o + 1) * 512])
                        qs = xps.tile([128, 512], F32, name=f"qs{o}_{s}", tag="qs")
                        nc.tensor.matmul(qs, perm_sb, raw, start=True, stop=True)
                        t1 = epool.tile([128, 512], F32, name=f"t1{o}_{s}", tag="t1")
                        nc.vector.tensor_mul(t1, raw, ropes[cn])
                        t2 = epool.tile([128, 512], F32, name=f"t2{o}_{s}", tag="t2")
                        nc.vector.tensor_mul(t2, qs, ropes[bn])
                        t3 = epool.tile([128, 512], F32, name=f"t3{o}_{s}", tag="t3")
                        nc.vector.tensor_add(t3, t1, t2)
                        qfin = epool.tile([128, 512], F32R, name=f"qf{o}_{s}", tag="qfin")
                        nc.vector.tensor_mul(qfin, t3, rb)
                        nc.gpsimd.dma_start(qk_d[o * 128:(o + 1) * 128, sl], qfin)

                    # V: transpose to natural [s, d] layout
                    for b in range(4):
                        vt = xps.tile([128, 128], F32, name=f"vt_{s}_{b}", tag="ssq", bufs=2)
                        nc.tensor.transpose(vt, vraw[:, b * 128:(b + 1) * 128], ident_sb)
                        vblk = epool.tile([128, 128], F32R, name=f"vb_{s}_{b}", tag="vblk")
                        nc.vector.tensor_copy(vblk, vt)
                        r0 = (s * 4 + b) * 128
                        nc.gpsimd.dma_start(v_d[r0:r0 + 128, :], vblk)

            # ---------------- Phase 2+3: SDPA + output projections ---------
            with nc.named_scope("p2"), \
                 tc.tile_pool(name="kv", bufs=1) as kvpool, \
                 tc.tile_pool(name="qh", bufs=2) as qpool, \
                 tc.tile_pool(name="pb", bufs=3) as ppool, \
                 tc.tile_pool(name="sm", bufs=2) as smpool, \
                 tc.tile_pool(name="wo", bufs=1) as wopool, \
                 tc.tile_pool(name="ot", bufs=3) as otpool, \
                 tc.tile_pool(name="os", bufs=3) as ospool, \
                 tc.tile_pool(name="sc", bufs=2, space="PSUM") as scps, \
                 tc.tile_pool(name="oa", bufs=1, space="PSUM") as oaps:
                kT_sb = kvpool.tile([128, S], F32R)
                v_sb = kvpool.tile([128, NB, 128], F32R)
                v_r = v_d.rearrange("(t p) d -> p t d", p=128)
                for s in range(ST):
                    ssl = slice(s * 512, (s + 1) * 512)
                    nc.sync.dma_start(kT_sb[:, ssl], qk_d[3 * 128:4 * 128, ssl])
                    nc.sync.dma_start(v_sb[:, 4 * s:4 * s + 4, :], v_r[:, 4 * s:4 * s + 4, :])
                wout_sb = wopool.tile([128, HPC, D], F32R)
                nc.sync.dma_start(wout_sb, wout.rearrange("(hb p) n -> p hb n", p=128))
                waout_sb = wopool.tile([128, HPC, D], F32R)
                nc.sync.dma_start(waout_sb, waout.rearrange("(hb p) n -> p hb n", p=128))

                for h in range(HPC):
                    q_sb = qpool.tile([128, S], F32R, name=f"q_{h}", tag="qh")
                    for s in range(ST):
                        ssl = slice(s * 512, (s + 1) * 512)
                        nc.sync.dma_start(q_sb[:, ssl], qk_d[h * 128:(h + 1) * 128, ssl])
                    for s in range(ST):
                        sl = slice(s * 512, (s + 1) * 512)
                        outacc = oaps.tile([128, 512], F32, name=f"oa_{h}_{s}", tag="oa", bufs=1)
                        sumacc = oaps.tile([1, 512], F32, name=f"sa_{h}_{s}", tag="sa", bufs=1)
                        GK = 2
                        for g in range(NB // GK):
                            sc = scps.tile([128, GK * 512], F32, name=f"sc_{h}_{s}_{g}", tag="sc", bufs=3)
                            for j in range(GK):
                                t = g * GK + j
                                nc.tensor.matmul(
                                    sc[:, j * 512:(j + 1) * 512],
                                    kT_sb[:, t * 128:(t + 1) * 128],
                                    q_sb[:, sl],
                                    start=True, stop=True)
                            pb = ppool.tile([128, GK * 512], F32R, name=f"pb_{h}_{s}_{g}", tag="pb", bufs=3)
                            nc.scalar.activation(pb, sc, AF.Exp, scale=SCALE)
                            for j in range(GK):
                                t = g * GK + j
                                nc.tensor.matmul(
                                    outacc, v_sb[:, t, :],
                                    pb[:, j * 512:(j + 1) * 512],
                                    start=(t == 0), stop=(t == NB - 1))
                            pbf = ppool.tile([128, 512], F32R, name=f"pbf_{h}_{s}_{g}",
                                             tag="pbf", bufs=2)
                            nc.vector.tensor_add(pbf, pb[:, 0:512], pb[:, 512:1024])
                            nc.tensor.matmul(sumacc, ones_col, pbf,
                                             start=(g == 0), stop=(g == NB // GK - 1))
                        oevac = smpool.tile([128, 512], F32, name=f"oe_{h}_{s}", tag="oevac")
                        nc.vector.tensor_copy(oevac, outacc)
                        sum_sb = smpool.tile([1, 512], F32, name=f"su_{h}_{s}", tag="sum_sb")
                        nc.vector.tensor_copy(sum_sb, sumacc)
                        recip = smpool.tile([1, 512], F32, name=f"rc_{h}_{s}", tag="recip")
                        rscr = smpool.tile([1, 512], F32, name=f"rs_{h}_{s}", tag="rscr")
                        nc.vector.reciprocal_approx_accurate(recip, sum_sb, scratch=rscr)
                        rbs = smpool.tile([128, 512], F32, name=f"rbs_{h}_{s}", tag="rbs")
                        nc.gpsimd.partition_broadcast(rbs, recip)
                        ofin = smpool.tile([128, 512], F32R, name=f"of_{h}_{s}", tag="ofin")
                        nc.vector.tensor_mul(ofin, oevac, rbs)
                        nc.gpsimd.dma_start(oT_d[h * 128:(h + 1) * 128, sl], ofin)

                # ---- output projections (interleave with SDPA tail) ----
                oT_r = oT_d.rearrange("(hb p) s -> p hb s", p=128)
                with nc.named_scope("p3"):
                    for sb in range(NB):
                        wsel = waout_sb if sb < (S_TXT // 128) else wout_sb
                        ot = otpool.tile([128, HPC, 128], F32R, name=f"ot_{sb}", tag="ots")
                        nc.sync.dma_start(ot, oT_r[:, :, sb * 128:(sb + 1) * 128])
                        for n in range(D // 512):
                            # late tiles run after SDPA ends: rotate through the
                            # freed oa/sa banks for deeper psum pipelining
                            if sb >= 24:
                                tg = ("sc", "oa", "sa")[n % 3]
                                pool_ = scps if tg == "sc" else oaps
                                ps = pool_.tile([128, 512], F32, name=f"po_{sb}_{n}", tag=tg,
                                                bufs=3 if tg == "sc" else 1)
                            else:
                                ps = scps.tile([128, 512], F32, name=f"po_{sb}_{n}", tag="sc", bufs=3)
                            for hb in range(HPC):
                                nc.tensor.matmul(
                                    ps,
                                    ot[:, hb, :],
                                    wsel[:, hb, n * 512:(n + 1) * 512],
                                    start=(hb == 0), stop=(hb == HPC - 1))
                            osb = ospool.tile([128, 512], F32, name=f"os_{sb}_{n}", tag="os")
                            if n % 2 == 0:
                                nc.vector.tensor_copy(osb, ps)
                            else:
                                nc.scalar.copy(osb, ps)
                            nc.gpsimd.dma_start(
                                part[sb * 128:(sb + 1) * 128, n * 512:(n + 1) * 512], osb)

    nc.compile()
    return nc


def _prep_inputs(inputs):
    """Host-side shard prep: one in_map per core."""
    hs = np.asarray(inputs["hidden_states"], dtype=np.float32)[0]        # [4096, 3072]
    ehs = np.asarray(inputs["encoder_hidden_states"], dtype=np.float32)[0]  # [512, 3072]
    ire = np.asarray(inputs["image_rotary_emb"], dtype=np.float32)       # [2, 4608, 128]

    x = np.concatenate([ehs, hs], axis=0)                 # [S, D], text first
    xT = np.ascontiguousarray(x.T)                        # [D, S]

    cos, sin = ire[0], ire[1]                             # [S, 128]
    nq = np.asarray(inputs["nq"], np.float32)
    nk = np.asarray(inputs["nk"], np.float32)
    anq = np.asarray(inputs["anq"], np.float32)
    ank = np.asarray(inputs["ank"], np.float32)
    wq_rows = np.concatenate([np.broadcast_to(anq, (S_TXT, HD)),
                              np.broadcast_to(nq, (S_IMG, HD))], axis=0)  # [S,128]
    wk_rows = np.concatenate([np.broadcast_to(ank, (S_TXT, HD)),
                              np.broadcast_to(nk, (S_IMG, HD))], axis=0)

    def rope_cb(w_rows):
        C = w_rows * cos
        B = np.empty_like(C)
        B[:, 0::2] = -sin[:, 0::2] * w_rows[:, 1::2]
        B[:, 1::2] = sin[:, 1::2] * w_rows[:, 0::2]
        return (np.ascontiguousarray(C.T), np.ascontiguousarray(B.T))

    Cq, Bq = rope_cb(wq_rows)
    Ck, Bk = rope_cb(wk_rows)

    perm = np.zeros((128, 128), np.float32)
    idx = np.arange(0, 128, 2)
    perm[idx, idx + 1] = 1.0
    perm[idx + 1, idx] = 1.0
    ident = np.eye(128, dtype=np.float32)

    Wq, Wk, Wv = inputs["Wq"], inputs["Wk"], inputs["Wv"]
    aWq, aWk, aWv = inputs["aWq"], inputs["aWk"], inputs["aWv"]
    bq, bk, bv = inputs["bq"], inputs["bk"], inputs["bv"]
    abq, abk, abv = inputs["abq"], inputs["abk"], inputs["abv"]
    Wout, Waout = inputs["Wout"], inputs["Waout"]

    in_maps = []
    for i in range(NCORES):
        qs = slice(i * 384, (i + 1) * 384)
        ks = slice(i * 128, (i + 1) * 128)
        wimg_i = np.ascontiguousarray(np.concatenate(
            [Wq[:, qs], Wk[:, ks], Wv[:, ks]], axis=1), dtype=np.float32)
        wtxt_i = np.ascontiguousarray(np.concatenate(
            [aWq[:, qs], aWk[:, ks], aWv[:, ks]], axis=1), dtype=np.float32)
        bias_i = np.stack([
            np.concatenate([abq[qs].reshape(3, 128), abk[ks][None], abv[ks][None]]),
            np.concatenate([bq[qs].reshape(3, 128), bk[ks][None], bv[ks][None]]),
        ]).astype(np.float32)                              # [2, 5, 128]
        in_maps.append({
            "xT": xT,
            "wtxt": wtxt_i,
            "wimg": wimg_i,
            "biasd": bias_i,
            "ropeCq": Cq, "ropeBq": Bq, "ropeCk": Ck, "ropeBk": Bk,
            "permd": perm, "identd": ident,
            "wout": np.ascontiguousarray(Wout[qs, :], dtype=np.float32),
            "waout": np.ascontiguousarray(Waout[qs, :], dtype=np.float32),
        })
    return in_maps


def kernel(**inputs):
    from concourse.bass_utils import run_bass_kernel_spmd

    if "nc" not in _CACHE:
        _CACHE["nc"] = _build()
    nc = _CACHE["nc"]

    in_maps = _prep_inputs(inputs)
    res = None
    last_err = None
    for _attempt in range(3):
        try:
            res = run_bass_kernel_spmd(nc, in_maps, core_ids=list(range(NCORES)))
            break
        except Exception as e:  # transient device wedges (NRT_EXEC_UNIT_*)
            last_err = e
            try:
                import jax
                jax.clear_backends()
            except Exception:
                pass
    if res is None:
        raise last_err
    total = res.results[0]["part"].astype(np.float64)
    for r in res.results[1:]:
        total += r["part"].astype(np.float64)

    enc = total[:S_TXT] + np.asarray(inputs["baout"], np.float64)
    hid = total[S_TXT:] + np.asarray(inputs["bout"], np.float64)
    return (hid[None].astype(np.float32), enc[None].astype(np.float32))
